# revision 3
# baseline (speedup 1.0000x reference)
"""Trainium2 Bass kernel for nn_AutoEncoder (GRU autoencoder).

Model: B=512,T=28,V=32000,E=256,H=D=512,L=128.
Sharding: pure data parallelism over batch: 8 cores x 64 rows, weights
replicated, no collectives. Host does the embedding gather + weight
packing/transposes; the device kernel computes encoder GRU, context
selection, fc1/fc2, decoder GRU, re_emb and the big hidden->vocab
matmul ([1792,512]@[512,32000] per core).

All matmuls run in fp32r (TensorE fast fp32 mode, ~1e-4 relmax). Biases
enter PSUM via K=1 ones-row matmuls. Gate math in B-layout
[64 partitions, features]; per-step h is transposed back to [512,64]
chunks with PE transposes so it can be the next matmul's stationary
operand.
"""

import sys

if "/opt/trn_rl_repo" not in sys.path:
    sys.path.insert(0, "/opt/trn_rl_repo")

import numpy as np

import concourse.bass as bass
import concourse.mybir as mybir
import concourse.tile as tile
from concourse import bacc
from concourse.bass_utils import run_bass_kernel_spmd
from concourse.masks import make_identity

F32 = mybir.dt.float32
F32R = mybir.dt.float32r
AF = mybir.ActivationFunctionType
OP = mybir.AluOpType

N_CORES = 8
B, T, V, E, H, D, L = 512, 28, 32000, 256, 512, 512, 128
BS = B // N_CORES          # 64 batch rows per core
TB = T * BS                # 1792
NEG_SLOPE = 0.02

# vocab chunking for the hidden->vocab matmul
VCH = 1024                 # columns per psum group (2 banks)
V_CHUNKS = [(c, min(VCH, V - c)) for c in range(0, V, VCH)]

_COMPILED = {}


def _input_specs():
    return {
        "xembT": ([E, TB], F32R),
        "xlastT": ([E, BS], F32R),
        "mask": ([BS, T], F32),
        "Wrz_enc": ([H + E, 1024], F32R),
        "Wnx_enc": ([E, 512], F32R),
        "Wnh_enc": ([H, 512], F32R),
        "brz_enc": ([1, 1024], F32R),
        "binn_enc": ([1, 512], F32R),
        "bhn_enc": ([1, 512], F32R),
        "Wb_enc": ([E, 1536], F32R),
        "bb_enc": ([1, 1536], F32R),
        "bhn_b": ([BS, 512], F32),
        "fc1T": ([2 * H, L], F32R),
        "fc1b": ([1, L], F32R),
        "fc2T": ([L, D], F32R),
        "fc2b": ([1, D], F32R),
        "WihdT": ([D, 3 * D], F32R),
        "bihd": ([1, 3 * D], F32R),
        "Wrz_dec": ([D, 1024], F32R),
        "Wnh_dec": ([D, 512], F32R),
        "brz_dec": ([1, 1024], F32R),
        "bhn_dec": ([1, 512], F32R),
        "rewT": ([D, E], F32R),
        "reb": ([1, E], F32R),
        "hvT": ([D, V], F32R),
        "hvb": ([1, V], F32R),
    }


def build_nc():
    nc = bacc.Bacc("TRN2", target_bir_lowering=False, debug=False)

    din = {}
    for name, (shape, dt) in _input_specs().items():
        din[name] = nc.dram_tensor(name, shape, dt, kind="ExternalInput").ap()

    logits_o = nc.dram_tensor("logits_o", [TB, V], F32, kind="ExternalOutput").ap()
    z_o = nc.dram_tensor("z_o", [BS, L], F32, kind="ExternalOutput").ap()
    re_o = nc.dram_tensor("re_o", [TB, E], F32, kind="ExternalOutput").ap()

    with tile.TileContext(nc) as tc:
        _build_body(nc, tc, din, logits_o, z_o, re_o)

    nc.compile()
    return nc


def _build_body(nc, tc, din, logits_o, z_o, re_o):
    from contextlib import ExitStack

    sigm, tanh = AF.Sigmoid, AF.Tanh

    with ExitStack() as ctx:
        persist = ctx.enter_context(tc.tile_pool(name="persist", bufs=1))

        ident = persist.tile([128, 128], F32)
        make_identity(nc, ident[:])
        ones_f = persist.tile([1, 128], F32)
        nc.vector.memset(ones_f[:], 1.0)
        ones128 = persist.tile([1, 128], F32R)
        nc.vector.tensor_copy(ones128[:], ones_f[:])
        ones = ones128[:, :BS]

        mask_t = persist.tile([BS, T], F32)
        nc.sync.dma_start(mask_t[:], din["mask"])
        outs_T = persist.tile([128, 4, TB], F32R)   # decoder outputs, T-layout
        ctx_acc = persist.tile([BS, H], F32)
        gi_dec = persist.tile([BS, 3 * D], F32)
        ctxT = persist.tile([128, 8, BS], F32R)
        out0T = persist.tile([128, 4, BS], F32R)

        with ExitStack() as ph1:
            gates_ps = ph1.enter_context(
                tc.tile_pool(name="gates_ps", bufs=1, space="PSUM"))
            tp_ps = ph1.enter_context(
                tc.tile_pool(name="tp_ps", bufs=4, space="PSUM"))
            gate_sb = ph1.enter_context(tc.tile_pool(name="gate_sb", bufs=2))
            h_sb = ph1.enter_context(tc.tile_pool(name="h_sb", bufs=2))
            hT_sb = ph1.enter_context(tc.tile_pool(name="hT_sb", bufs=2))

            def bias_mm(psum_slice, brow, stop=False):
                nc.tensor.matmul(psum_slice, ones, brow, start=True, stop=stop)

            def transpose_to(dsts, src_b):
                """src_b [BS, n*128] f32 -> dsts[k] [128, BS] f32r via PE."""
                for k, dst in enumerate(dsts):
                    tp = tp_ps.tile([128, BS], F32, tag="tp")
                    nc.tensor.transpose(
                        tp[:], src_b[:, k * 128:(k + 1) * 128], ident[:BS, :BS]
                    )
                    nc.scalar.copy(dst, tp[:])

            # ================= encoder =================
            with tc.tile_pool(name="enc_w", bufs=1) as enc_w:
                xembT = enc_w.tile([128, 2, TB], F32R)
                for k in range(2):
                    nc.sync.dma_start(
                        xembT[:, k, :], din["xembT"][k * 128:(k + 1) * 128, :])
                xlastT = enc_w.tile([128, 2, BS], F32R)
                for k in range(2):
                    nc.sync.dma_start(
                        xlastT[:, k, :], din["xlastT"][k * 128:(k + 1) * 128, :])
                Wrz = enc_w.tile([128, 6, 1024], F32R)
                for k in range(6):
                    nc.sync.dma_start(
                        Wrz[:, k, :], din["Wrz_enc"][k * 128:(k + 1) * 128, :])
                Wnx = enc_w.tile([128, 2, 512], F32R)
                for k in range(2):
                    nc.sync.dma_start(
                        Wnx[:, k, :], din["Wnx_enc"][k * 128:(k + 1) * 128, :])
                Wnh = enc_w.tile([128, 4, 512], F32R)
                for k in range(4):
                    nc.sync.dma_start(
                        Wnh[:, k, :], din["Wnh_enc"][k * 128:(k + 1) * 128, :])
                brz = enc_w.tile([1, 1024], F32R)
                nc.sync.dma_start(brz[:], din["brz_enc"])
                binn = enc_w.tile([1, 512], F32R)
                nc.sync.dma_start(binn[:], din["binn_enc"])
                bhn = enc_w.tile([1, 512], F32R)
                nc.sync.dma_start(bhn[:], din["bhn_enc"])
                Wb = enc_w.tile([128, 2, 1536], F32R)
                for k in range(2):
                    nc.sync.dma_start(
                        Wb[:, k, :], din["Wb_enc"][k * 128:(k + 1) * 128, :])
                bb = enc_w.tile([1, 1536], F32R)
                nc.sync.dma_start(bb[:], din["bb_enc"])
                bhn_b = enc_w.tile([BS, 512], F32)
                nc.sync.dma_start(bhn_b[:], din["bhn_b"])

                h_B = None
                h_T = None
                for t in range(T):
                    # psum banks: [0:1024] rz, [1024:1536] inn, [1536:2048] hn
                    P = gates_ps.tile([BS, 2048], F32, tag="g")
                    bias_mm(P[:, 0:512], brz[:, 0:512])
                    bias_mm(P[:, 512:1024], brz[:, 512:1024])
                    bias_mm(P[:, 1024:1536], binn[:])
                    bias_mm(P[:, 1536:2048], bhn[:], stop=(t == 0))
                    xsl = slice(t * BS, (t + 1) * BS)
                    for kk in range(2):  # x contributions
                        for nn in range(2):
                            nc.tensor.matmul(
                                P[:, nn * 512:(nn + 1) * 512],
                                xembT[:, kk, xsl],
                                Wrz[:, 4 + kk, nn * 512:(nn + 1) * 512],
                                start=False, stop=(t == 0 and kk == 1),
                            )
                        nc.tensor.matmul(
                            P[:, 1024:1536], xembT[:, kk, xsl], Wnx[:, kk, :],
                            start=False, stop=(kk == 1),
                        )
                    if t > 0:
                        for k in range(4):  # h contributions
                            for nn in range(2):
                                nc.tensor.matmul(
                                    P[:, nn * 512:(nn + 1) * 512],
                                    h_T[k], Wrz[:, k, nn * 512:(nn + 1) * 512],
                                    start=False, stop=(k == 3),
                                )
                            nc.tensor.matmul(
                                P[:, 1536:2048], h_T[k], Wnh[:, k, :],
                                start=False, stop=(k == 3),
                            )

                    rz = gate_sb.tile([BS, 1024], F32, tag="rz")
                    nc.scalar.activation(rz[:], P[:, 0:1024], sigm)
                    t1 = gate_sb.tile([BS, 512], F32, tag="t1")
                    nc.vector.tensor_mul(t1[:], rz[:, 0:512], P[:, 1536:2048])
                    t2 = gate_sb.tile([BS, 512], F32, tag="t2")
                    nc.vector.tensor_add(t2[:], t1[:], P[:, 1024:1536])
                    n_sb = gate_sb.tile([BS, 512], F32, tag="n")
                    nc.scalar.activation(n_sb[:], t2[:], tanh)

                    h_new = h_sb.tile([BS, H], F32, tag="h")
                    if t == 0:
                        # h' = (1-z) * n
                        zm1 = gate_sb.tile([BS, 512], F32, tag="d")
                        nc.vector.tensor_scalar(
                            zm1[:], rz[:, 512:1024], -1.0, 1.0,
                            op0=OP.mult, op1=OP.add)
                        nc.vector.tensor_mul(h_new[:], zm1[:], n_sb[:])
                    else:
                        d = gate_sb.tile([BS, 512], F32, tag="d")
                        nc.vector.tensor_sub(d[:], h_B[:], n_sb[:])
                        e = gate_sb.tile([BS, 512], F32, tag="e")
                        nc.vector.tensor_mul(e[:], rz[:, 512:1024], d[:])
                        nc.vector.tensor_add(h_new[:], n_sb[:], e[:])

                    # ctx_f accumulation: ctx += mask[:, t] * h
                    msel = gate_sb.tile([BS, 512], F32, tag="msel")
                    nc.scalar.activation(
                        msel[:], h_new[:], AF.Identity, scale=mask_t[:, t:t + 1])
                    if t == 0:
                        nc.vector.tensor_copy(ctx_acc[:], msel[:])
                    else:
                        nc.vector.tensor_add(ctx_acc[:], ctx_acc[:], msel[:])

                    if t < T - 1:
                        hT = hT_sb.tile([128, 4, BS], F32R, tag="hT")
                        transpose_to([hT[:, k, :] for k in range(4)], h_new[:])
                        h_T = [hT[:, k, :] for k in range(4)]
                    h_B = h_new

                # ---------- ctx_b: single GRU step on last token, h0=0 -------
                PB = gates_ps.tile([BS, 1536], F32, tag="g")
                bias_mm(PB[:, 0:512], bb[:, 0:512])
                bias_mm(PB[:, 512:1024], bb[:, 512:1024])
                bias_mm(PB[:, 1024:1536], bb[:, 1024:1536])
                for kk in range(2):
                    for nn in range(3):
                        nc.tensor.matmul(
                            PB[:, nn * 512:(nn + 1) * 512],
                            xlastT[:, kk, :], Wb[:, kk, nn * 512:(nn + 1) * 512],
                            start=False, stop=(kk == 1),
                        )
                rzb = gate_sb.tile([BS, 1024], F32, tag="rz")
                nc.scalar.activation(rzb[:], PB[:, 0:1024], sigm)
                t1b = gate_sb.tile([BS, 512], F32, tag="t1")
                nc.vector.tensor_mul(t1b[:], rzb[:, 0:512], bhn_b[:])
                t2b = gate_sb.tile([BS, 512], F32, tag="t2")
                nc.vector.tensor_add(t2b[:], t1b[:], PB[:, 1024:1536])
                nb = gate_sb.tile([BS, 512], F32, tag="n")
                nc.scalar.activation(nb[:], t2b[:], tanh)
                zm1b = gate_sb.tile([BS, 512], F32, tag="d")
                nc.vector.tensor_scalar(
                    zm1b[:], rzb[:, 512:1024], -1.0, 1.0, op0=OP.mult, op1=OP.add)
                ctx_b = h_sb.tile([BS, H], F32, tag="h")
                nc.vector.tensor_mul(ctx_b[:], zm1b[:], nb[:])

                # context_T [1024, 64] chunks: ctx_f (4) then ctx_b (4)
                transpose_to([ctxT[:, k, :] for k in range(4)], ctx_acc[:])
                transpose_to([ctxT[:, 4 + k, :] for k in range(4)], ctx_b[:])

            # ================= fc1 / fc2 / gi_dec / decoder / re_emb ========
            with tc.tile_pool(name="dec_w", bufs=1) as dec_w:
                fc1T_t = dec_w.tile([128, 8, L], F32R)
                for k in range(8):
                    nc.sync.dma_start(
                        fc1T_t[:, k, :], din["fc1T"][k * 128:(k + 1) * 128, :])
                fc1b_t = dec_w.tile([1, L], F32R)
                nc.sync.dma_start(fc1b_t[:], din["fc1b"])
                fc2T_t = dec_w.tile([128, D], F32R)
                nc.sync.dma_start(fc2T_t[:], din["fc2T"])
                fc2b_t = dec_w.tile([1, D], F32R)
                nc.sync.dma_start(fc2b_t[:], din["fc2b"])
                WihdT_t = dec_w.tile([128, 4, 3 * D], F32R)
                for k in range(4):
                    nc.sync.dma_start(
                        WihdT_t[:, k, :], din["WihdT"][k * 128:(k + 1) * 128, :])
                bihd_t = dec_w.tile([1, 3 * D], F32R)
                nc.sync.dma_start(bihd_t[:], din["bihd"])
                Wrzd = dec_w.tile([128, 4, 1024], F32R)
                for k in range(4):
                    nc.sync.dma_start(
                        Wrzd[:, k, :], din["Wrz_dec"][k * 128:(k + 1) * 128, :])
                Wnhd = dec_w.tile([128, 4, 512], F32R)
                for k in range(4):
                    nc.sync.dma_start(
                        Wnhd[:, k, :], din["Wnh_dec"][k * 128:(k + 1) * 128, :])
                brzd = dec_w.tile([1, 1024], F32R)
                nc.sync.dma_start(brzd[:], din["brz_dec"])
                bhnd = dec_w.tile([1, 512], F32R)
                nc.sync.dma_start(bhnd[:], din["bhn_dec"])
                rewT_t = dec_w.tile([128, 4, E], F32R)
                for k in range(4):
                    nc.sync.dma_start(
                        rewT_t[:, k, :], din["rewT"][k * 128:(k + 1) * 128, :])
                reb_t = dec_w.tile([1, E], F32R)
                nc.sync.dma_start(reb_t[:], din["reb"])

                # z = context @ fc1_w.T + fc1_b
                PZ = gates_ps.tile([BS, L], F32, tag="g")
                bias_mm(PZ[:], fc1b_t[:])
                for k in range(8):
                    nc.tensor.matmul(
                        PZ[:], ctxT[:, k, :], fc1T_t[:, k, :],
                        start=False, stop=(k == 7))
                z_sb = gate_sb.tile([BS, L], F32, tag="zz")
                nc.vector.tensor_copy(z_sb[:], PZ[:])
                nc.sync.dma_start(z_o, z_sb[:])
                zT_ps = tp_ps.tile([128, BS], F32, tag="tp")
                nc.tensor.transpose(zT_ps[:L, :], z_sb[:], ident[:BS, :BS])
                zT = hT_sb.tile([L, BS], F32R, tag="zT")
                nc.scalar.copy(zT[:], zT_ps[:L, :])

                # out0 = z @ fc2_w.T + fc2_b
                PO = gates_ps.tile([BS, D], F32, tag="g")
                bias_mm(PO[:], fc2b_t[:])
                nc.tensor.matmul(PO[:], zT[:], fc2T_t[:], start=False, stop=True)
                out0_B = h_sb.tile([BS, D], F32, tag="h")
                nc.vector.tensor_copy(out0_B[:], PO[:])
                transpose_to([out0T[:, k, :] for k in range(4)], out0_B[:])

                # gi_dec = out0 @ dec_Wih.T + dec_bih
                PG = gates_ps.tile([BS, 3 * D], F32, tag="g")
                for nn in range(3):
                    bias_mm(PG[:, nn * 512:(nn + 1) * 512],
                            bihd_t[:, nn * 512:(nn + 1) * 512])
                for k in range(4):
                    for nn in range(3):
                        nc.tensor.matmul(
                            PG[:, nn * 512:(nn + 1) * 512],
                            out0T[:, k, :], WihdT_t[:, k, nn * 512:(nn + 1) * 512],
                            start=False, stop=(k == 3),
                        )
                nc.scalar.copy(gi_dec[:], PG[:])

                # ---------------- decoder ----------------
                h_B = out0_B
                h_T = [out0T[:, k, :] for k in range(4)]
                for t in range(T):
                    PD = gates_ps.tile([BS, 1536], F32, tag="g")
                    bias_mm(PD[:, 0:512], brzd[:, 0:512])
                    bias_mm(PD[:, 512:1024], brzd[:, 512:1024])
                    bias_mm(PD[:, 1024:1536], bhnd[:])
                    for k in range(4):
                        for nn in range(2):
                            nc.tensor.matmul(
                                PD[:, nn * 512:(nn + 1) * 512],
                                h_T[k], Wrzd[:, k, nn * 512:(nn + 1) * 512],
                                start=False, stop=(k == 3),
                            )
                        nc.tensor.matmul(
                            PD[:, 1024:1536], h_T[k], Wnhd[:, k, :],
                            start=False, stop=(k == 3),
                        )
                    rzp = gate_sb.tile([BS, 1024], F32, tag="rzp")
                    nc.vector.tensor_add(rzp[:], gi_dec[:, 0:1024], PD[:, 0:1024])
                    rz = gate_sb.tile([BS, 1024], F32, tag="rz")
                    nc.scalar.activation(rz[:], rzp[:], sigm)
                    t1 = gate_sb.tile([BS, 512], F32, tag="t1")
                    nc.vector.tensor_mul(t1[:], rz[:, 0:512], PD[:, 1024:1536])
                    t2 = gate_sb.tile([BS, 512], F32, tag="t2")
                    nc.vector.tensor_add(t2[:], t1[:], gi_dec[:, 1024:1536])
                    n_sb = gate_sb.tile([BS, 512], F32, tag="n")
                    nc.scalar.activation(n_sb[:], t2[:], tanh)
                    d = gate_sb.tile([BS, 512], F32, tag="d")
                    nc.vector.tensor_sub(d[:], h_B[:], n_sb[:])
                    e = gate_sb.tile([BS, 512], F32, tag="e")
                    nc.vector.tensor_mul(e[:], rz[:, 512:1024], d[:])
                    h_new = h_sb.tile([BS, D], F32, tag="h")
                    nc.vector.tensor_add(h_new[:], n_sb[:], e[:])

                    tsl = slice(t * BS, (t + 1) * BS)
                    transpose_to([outs_T[:, k, tsl] for k in range(4)], h_new[:])
                    h_T = [outs_T[:, k, tsl] for k in range(4)]
                    h_B = h_new

                # ---------------- re_emb ----------------
                for m in range(TB // 128):
                    msl = slice(m * 128, (m + 1) * 128)
                    PR = gates_ps.tile([128, E], F32, tag="g")
                    nc.tensor.matmul(
                        PR[:], ones128[:], reb_t[:], start=True, stop=False)
                    for k in range(4):
                        nc.tensor.matmul(
                            PR[:], outs_T[:, k, msl], rewT_t[:, k, :],
                            start=False, stop=(k == 3),
                        )
                    # exact leaky relu: max(x, alpha*x)
                    re_a = gate_sb.tile([128, E], F32, tag="rea")
                    nc.vector.tensor_scalar_mul(re_a[:], PR[:], NEG_SLOPE)
                    re_sb = gate_sb.tile([128, E], F32, tag="re")
                    nc.vector.tensor_max(re_sb[:], PR[:], re_a[:])
                    nc.sync.dma_start(re_o[msl, :], re_sb[:])

        # ================= logits =================
        with (
            tc.tile_pool(name="hv_w", bufs=3) as hv_w,
            tc.tile_pool(name="stage", bufs=4) as stage_p,
            tc.tile_pool(name="lg_ps", bufs=2, space="PSUM") as lg_ps,
        ):
            for c0, clen in V_CHUNKS:
                hv_t = hv_w.tile([128, 4, VCH], F32R, tag="hv")
                for k in range(4):
                    nc.sync.dma_start(
                        hv_t[:, k, :clen],
                        din["hvT"][k * 128:(k + 1) * 128, c0:c0 + clen])
                hvb_t = hv_w.tile([1, VCH], F32R, tag="hvb")
                nc.sync.dma_start(hvb_t[:, :clen], din["hvb"][:, c0:c0 + clen])
                for m in range(TB // 128):
                    msl = slice(m * 128, (m + 1) * 128)
                    P = lg_ps.tile([128, VCH], F32, tag="lg")
                    for nt in range(0, clen, 512):
                        ntl = min(512, clen - nt)
                        nc.tensor.matmul(
                            P[:, nt:nt + ntl], ones128[:], hvb_t[:, nt:nt + ntl],
                            start=True, stop=False,
                        )
                        for k in range(4):
                            nc.tensor.matmul(
                                P[:, nt:nt + ntl],
                                outs_T[:, k, msl], hv_t[:, k, nt:nt + ntl],
                                start=False, stop=(k == 3),
                            )
                    st = stage_p.tile([128, VCH], F32, tag="st")
                    if m % 2 == 0:
                        nc.scalar.copy(st[:, :clen], P[:, :clen])
                    else:
                        nc.vector.tensor_copy(st[:, :clen], P[:, :clen])
                    nc.sync.dma_start(logits_o[msl, c0:c0 + clen], st[:, :clen])


def _pack_host(inputs):
    """Full-model host prep. Returns per-core input dicts."""
    f32 = np.float32
    emb = np.asarray(inputs["emb"], f32)
    x = np.asarray(inputs["x"])
    x_lens = np.asarray(inputs["x_lens"])

    WihfT = np.ascontiguousarray(np.asarray(inputs["enc_Wih_f"], f32).T)  # [E,3H]
    WhhfT = np.ascontiguousarray(np.asarray(inputs["enc_Whh_f"], f32).T)  # [H,3H]
    bihf = np.asarray(inputs["enc_bih_f"], f32)
    bhhf = np.asarray(inputs["enc_bhh_f"], f32)
    WihbT = np.ascontiguousarray(np.asarray(inputs["enc_Wih_b"], f32).T)
    bihb = np.asarray(inputs["enc_bih_b"], f32)
    bhhb = np.asarray(inputs["enc_bhh_b"], f32)
    fc1_w = np.asarray(inputs["fc1_w"], f32)
    fc1_b = np.asarray(inputs["fc1_b"], f32)
    fc2_w = np.asarray(inputs["fc2_w"], f32)
    fc2_b = np.asarray(inputs["fc2_b"], f32)
    WihdT = np.ascontiguousarray(np.asarray(inputs["dec_Wih"], f32).T)
    WhhdT = np.ascontiguousarray(np.asarray(inputs["dec_Whh"], f32).T)
    bihd = np.asarray(inputs["dec_bih"], f32)
    bhhd = np.asarray(inputs["dec_bhh"], f32)
    rewT = np.ascontiguousarray(np.asarray(inputs["re_w"], f32).T)
    re_b = np.asarray(inputs["re_b"], f32)
    hvT = np.ascontiguousarray(np.asarray(inputs["hv_w"], f32).T)
    hv_b = np.asarray(inputs["hv_b"], f32)

    shared = {
        "Wrz_enc": np.vstack([WhhfT[:, :1024], WihfT[:, :1024]]),
        "Wnx_enc": WihfT[:, 1024:],
        "Wnh_enc": WhhfT[:, 1024:],
        "brz_enc": (bihf + bhhf)[None, :1024],
        "binn_enc": bihf[None, 1024:],
        "bhn_enc": bhhf[None, 1024:],
        "Wb_enc": WihbT,
        "bb_enc": (bihb + np.concatenate([bhhb[:1024], np.zeros(512, f32)]))[None],
        "bhn_b": np.tile(bhhb[None, 1024:], (BS, 1)),
        "fc1T": fc1_w.T,
        "fc1b": fc1_b[None],
        "fc2T": fc2_w.T,
        "fc2b": fc2_b[None],
        "WihdT": WihdT,
        "bihd": bihd[None],
        "Wrz_dec": WhhdT[:, :1024],
        "Wnh_dec": WhhdT[:, 1024:],
        "brz_dec": bhhd[None, :1024],
        "bhn_dec": bhhd[None, 1024:],
        "rewT": rewT,
        "reb": re_b[None],
        "hvT": hvT,
        "hvb": hv_b[None],
    }
    shared = {k: np.ascontiguousarray(v, dtype=f32) for k, v in shared.items()}

    idx = (np.asarray(x_lens) - 1).astype(np.int64)
    per_core = []
    for s in range(N_CORES):
        rows = slice(s * BS, (s + 1) * BS)
        xs = np.asarray(x[rows], np.int64)
        emb_s = emb[xs]                    # [BS, T, E]
        xembT = np.ascontiguousarray(
            emb_s.transpose(2, 1, 0).reshape(E, TB), dtype=f32)
        idx_s = idx[rows]
        xlastT = np.ascontiguousarray(
            emb_s[np.arange(BS), idx_s].T, dtype=f32)      # [E, BS]
        mask = np.zeros((BS, T), f32)
        mask[np.arange(BS), idx_s] = 1.0
        d = dict(shared)
        d.update(xembT=xembT, xlastT=xlastT, mask=mask)
        per_core.append(d)
    return per_core


def kernel(**inputs):
    if "nc" not in _COMPILED:
        _COMPILED["nc"] = build_nc()
    nc = _COMPILED["nc"]

    in_maps = _pack_host(inputs)
    res = run_bass_kernel_spmd(nc, in_maps, core_ids=list(range(N_CORES)))

    logits = np.concatenate(
        [r["logits_o"].reshape(T, BS, V) for r in res.results], axis=1)
    z = np.concatenate([r["z_o"] for r in res.results], axis=0)
    re_emb = np.concatenate(
        [r["re_o"].reshape(T, BS, E).transpose(1, 0, 2) for r in res.results],
        axis=0)
    return logits, z, re_emb


# revision 6
# speedup vs baseline: 2.9155x; 2.9155x over previous
"""Trainium2 Bass kernel for nn_AutoEncoder (GRU autoencoder).

Model: B=512,T=28,V=32000,E=256,H=D=512,L=128.
Sharding: pure data parallelism over batch: 8 cores x 64 rows, weights
replicated, no collectives. Host does the embedding gather + weight
packing/transposes; the device kernel computes encoder GRU, context
selection, fc1/fc2, decoder GRU, re_emb and the big hidden->vocab
matmul ([1792,512]@[512,32000] per core).

All matmuls run in fp32r (TensorE fast fp32 mode, ~1e-4 relmax). Biases
enter PSUM via K=1 ones-row matmuls. Gate math in B-layout
[64 partitions, features]; per-step h is transposed back to [512,64]
chunks with PE transposes so it can be the next matmul's stationary
operand.
"""

import sys

if "/opt/trn_rl_repo" not in sys.path:
    sys.path.insert(0, "/opt/trn_rl_repo")

import numpy as np

import concourse.bass as bass
import concourse.mybir as mybir
import concourse.tile as tile
from concourse import bacc
from concourse.bass_utils import run_bass_kernel_spmd
from concourse.masks import make_identity

F32 = mybir.dt.float32
F32R = mybir.dt.float32r
AF = mybir.ActivationFunctionType
OP = mybir.AluOpType

N_CORES = 8
B, T, V, E, H, D, L = 512, 28, 32000, 256, 512, 512, 128
BS = B // N_CORES          # 64 batch rows per core
TB = T * BS                # 1792
NEG_SLOPE = 0.02

# vocab chunking for the hidden->vocab matmul
VCH = 1024                 # columns per psum group (2 banks)
V_CHUNKS = [(c, min(VCH, V - c)) for c in range(0, V, VCH)]

_COMPILED = {}


def _input_specs():
    return {
        "xembT": ([E, TB], F32R),
        "xlastT": ([E, BS], F32R),
        "mask": ([BS, T], F32),
        "Wrz_enc": ([H + E, 1024], F32R),
        "Wnx_enc": ([E, 512], F32R),
        "Wnh_enc": ([H, 512], F32R),
        "brz_enc": ([1, 1024], F32R),
        "binn_enc": ([1, 512], F32R),
        "bhn_enc": ([1, 512], F32R),
        "Wb_enc": ([E, 1536], F32R),
        "bb_enc": ([1, 1536], F32R),
        "bhn_b": ([BS, 512], F32),
        "fc1T": ([2 * H, L], F32R),
        "fc1b": ([1, L], F32R),
        "fc2T": ([L, D], F32R),
        "fc2b": ([1, D], F32R),
        "WihdT": ([D, 3 * D], F32R),
        "bihd": ([1, 3 * D], F32R),
        "Wrz_dec": ([D, 1024], F32R),
        "Wnh_dec": ([D, 512], F32R),
        "brz_dec": ([1, 1024], F32R),
        "bhn_dec": ([1, 512], F32R),
        "rewT": ([D, E], F32R),
        "reb": ([1, E], F32R),
        "hvT": ([D, V], F32R),
        "hvb_bc": ([128, V], F32),
    }


def build_nc():
    nc = bacc.Bacc("TRN2", target_bir_lowering=False, debug=False)

    din = {}
    for name, (shape, dt) in _input_specs().items():
        din[name] = nc.dram_tensor(name, shape, dt, kind="ExternalInput").ap()

    logits_o = nc.dram_tensor("logits_o", [TB, V], F32, kind="ExternalOutput").ap()
    z_o = nc.dram_tensor("z_o", [BS, L], F32, kind="ExternalOutput").ap()
    re_o = nc.dram_tensor("re_o", [TB, E], F32, kind="ExternalOutput").ap()

    with tile.TileContext(nc) as tc:
        _build_body(nc, tc, din, logits_o, z_o, re_o)

    nc.compile()
    return nc


def _build_body(nc, tc, din, logits_o, z_o, re_o):
    from contextlib import ExitStack

    sigm, tanh = AF.Sigmoid, AF.Tanh

    with ExitStack() as ctx:
        persist = ctx.enter_context(tc.tile_pool(name="persist", bufs=1))

        ident = persist.tile([128, 128], F32)
        make_identity(nc, ident[:])
        ones_f = persist.tile([1, 128], F32)
        nc.vector.memset(ones_f[:], 1.0)
        ones128 = persist.tile([1, 128], F32R)
        nc.vector.tensor_copy(ones128[:], ones_f[:])
        ones = ones128[:, :BS]

        mask_t = persist.tile([BS, T], F32)
        nc.sync.dma_start(mask_t[:], din["mask"])
        outs_T = persist.tile([128, 4, TB], F32R)   # decoder outputs, T-layout
        ctx_acc = persist.tile([BS, H], F32)
        gi_dec = persist.tile([BS, 3 * D], F32)
        ctxT = persist.tile([128, 8, BS], F32R)
        out0T = persist.tile([128, 4, BS], F32R)
        import os
        phases = os.environ.get("BENCH_PHASES", "all")

        with ExitStack() as ph1:
            tp_ps = ph1.enter_context(
                tc.tile_pool(name="tp_ps", bufs=2, space="PSUM"))
            gate_sb = ph1.enter_context(tc.tile_pool(name="gate_sb", bufs=2))
            h_sb = ph1.enter_context(tc.tile_pool(name="h_sb", bufs=2))
            hT_sb = ph1.enter_context(tc.tile_pool(name="hT_sb", bufs=2))

            def bias_mm(psum_slice, brow, stop=False):
                nc.tensor.matmul(psum_slice, ones, brow, start=True, stop=stop)

            def transpose_copy(dst_view, src_b, nk=4):
                """src_b [BS, nk*128] f32 -> dst_view [128, nk, BS] f32r.

                All nk PE transposes land in one psum bank; one ACT copy
                drains them to SBUF (with the f32r cast)."""
                tp = tp_ps.tile([128, 4 * BS], F32, tag="tp")
                for k in range(nk):
                    nc.tensor.transpose(
                        tp[:, k * BS:(k + 1) * BS],
                        src_b[:, k * 128:(k + 1) * 128], ident[:BS, :BS]
                    )
                nc.scalar.copy(dst_view, tp[:, :nk * BS])

            # ================= encoder =================
            with (
                tc.tile_pool(name="enc_w", bufs=1) as enc_w,
                tc.tile_pool(name="enc_ps", bufs=1, space="PSUM") as enc_ps,
            ):
                xembT = enc_w.tile([128, 2, TB], F32R)
                for k in range(2):
                    nc.sync.dma_start(
                        xembT[:, k, :], din["xembT"][k * 128:(k + 1) * 128, :])
                xlastT = enc_w.tile([128, 2, BS], F32R)
                for k in range(2):
                    nc.sync.dma_start(
                        xlastT[:, k, :], din["xlastT"][k * 128:(k + 1) * 128, :])
                Wrz = enc_w.tile([128, 6, 1024], F32R)
                for k in range(6):
                    nc.sync.dma_start(
                        Wrz[:, k, :], din["Wrz_enc"][k * 128:(k + 1) * 128, :])
                Wnx = enc_w.tile([128, 2, 512], F32R)
                for k in range(2):
                    nc.sync.dma_start(
                        Wnx[:, k, :], din["Wnx_enc"][k * 128:(k + 1) * 128, :])
                Wnh = enc_w.tile([128, 4, 512], F32R)
                for k in range(4):
                    nc.sync.dma_start(
                        Wnh[:, k, :], din["Wnh_enc"][k * 128:(k + 1) * 128, :])
                brz = enc_w.tile([1, 1024], F32R)
                nc.sync.dma_start(brz[:], din["brz_enc"])
                binn = enc_w.tile([1, 512], F32R)
                nc.sync.dma_start(binn[:], din["binn_enc"])
                bhn = enc_w.tile([1, 512], F32R)
                nc.sync.dma_start(bhn[:], din["bhn_enc"])
                Wb = enc_w.tile([128, 2, 1536], F32R)
                for k in range(2):
                    nc.sync.dma_start(
                        Wb[:, k, :], din["Wb_enc"][k * 128:(k + 1) * 128, :])
                bb = enc_w.tile([1, 1536], F32R)
                nc.sync.dma_start(bb[:], din["bb_enc"])
                bhn_b = enc_w.tile([BS, 512], F32)
                nc.sync.dma_start(bhn_b[:], din["bhn_b"])

                h_B = None
                h_T = None
                for t in range(T):
                    # psum banks: [0:1024] rz, [1024:1536] inn, [1536:2048] hn
                    P = enc_ps.tile([BS, 2048], F32, tag="g")
                    bias_mm(P[:, 0:512], brz[:, 0:512])
                    bias_mm(P[:, 512:1024], brz[:, 512:1024])
                    bias_mm(P[:, 1024:1536], binn[:])
                    bias_mm(P[:, 1536:2048], bhn[:], stop=(t == 0))
                    xsl = slice(t * BS, (t + 1) * BS)
                    for kk in range(2):  # x contributions
                        for nn in range(2):
                            nc.tensor.matmul(
                                P[:, nn * 512:(nn + 1) * 512],
                                xembT[:, kk, xsl],
                                Wrz[:, 4 + kk, nn * 512:(nn + 1) * 512],
                                start=False, stop=(t == 0 and kk == 1),
                            )
                        nc.tensor.matmul(
                            P[:, 1024:1536], xembT[:, kk, xsl], Wnx[:, kk, :],
                            start=False, stop=(kk == 1),
                        )
                    if t > 0:
                        for k in range(4):  # h contributions
                            for nn in range(2):
                                nc.tensor.matmul(
                                    P[:, nn * 512:(nn + 1) * 512],
                                    h_T[k], Wrz[:, k, nn * 512:(nn + 1) * 512],
                                    start=False, stop=(k == 3),
                                )
                            nc.tensor.matmul(
                                P[:, 1536:2048], h_T[k], Wnh[:, k, :],
                                start=False, stop=(k == 3),
                            )

                    rz = gate_sb.tile([BS, 1024], F32, tag="rz")
                    nc.scalar.activation(rz[:], P[:, 0:1024], sigm)
                    t1 = gate_sb.tile([BS, 512], F32, tag="t1")
                    nc.vector.tensor_mul(t1[:], rz[:, 0:512], P[:, 1536:2048])
                    t2 = gate_sb.tile([BS, 512], F32, tag="t2")
                    nc.vector.tensor_add(t2[:], t1[:], P[:, 1024:1536])
                    n_sb = gate_sb.tile([BS, 512], F32, tag="n")
                    nc.scalar.activation(n_sb[:], t2[:], tanh)

                    h_new = h_sb.tile([BS, H], F32, tag="h")
                    if t == 0:
                        # h' = (1-z) * n
                        zm1 = gate_sb.tile([BS, 512], F32, tag="d")
                        nc.vector.tensor_scalar(
                            zm1[:], rz[:, 512:1024], -1.0, 1.0,
                            op0=OP.mult, op1=OP.add)
                        nc.vector.tensor_mul(h_new[:], zm1[:], n_sb[:])
                    else:
                        d = gate_sb.tile([BS, 512], F32, tag="d")
                        nc.vector.tensor_sub(d[:], h_B[:], n_sb[:])
                        e = gate_sb.tile([BS, 512], F32, tag="e")
                        nc.vector.tensor_mul(e[:], rz[:, 512:1024], d[:])
                        nc.vector.tensor_add(h_new[:], n_sb[:], e[:])

                    # ctx_f accumulation: ctx += mask[:, t] * h
                    msel = gate_sb.tile([BS, 512], F32, tag="msel")
                    nc.scalar.activation(
                        msel[:], h_new[:], AF.Identity, scale=mask_t[:, t:t + 1])
                    if t == 0:
                        nc.vector.tensor_copy(ctx_acc[:], msel[:])
                    else:
                        nc.vector.tensor_add(ctx_acc[:], ctx_acc[:], msel[:])

                    if t < T - 1:
                        hT = hT_sb.tile([128, 4, BS], F32R, tag="hT")
                        transpose_copy(hT[:], h_new[:])
                        h_T = [hT[:, k, :] for k in range(4)]
                    h_B = h_new

                # ---------- ctx_b: single GRU step on last token, h0=0 -------
                PB = enc_ps.tile([BS, 1536], F32, tag="g")
                bias_mm(PB[:, 0:512], bb[:, 0:512])
                bias_mm(PB[:, 512:1024], bb[:, 512:1024])
                bias_mm(PB[:, 1024:1536], bb[:, 1024:1536])
                for kk in range(2):
                    for nn in range(3):
                        nc.tensor.matmul(
                            PB[:, nn * 512:(nn + 1) * 512],
                            xlastT[:, kk, :], Wb[:, kk, nn * 512:(nn + 1) * 512],
                            start=False, stop=(kk == 1),
                        )
                rzb = gate_sb.tile([BS, 1024], F32, tag="rz")
                nc.scalar.activation(rzb[:], PB[:, 0:1024], sigm)
                t1b = gate_sb.tile([BS, 512], F32, tag="t1")
                nc.vector.tensor_mul(t1b[:], rzb[:, 0:512], bhn_b[:])
                t2b = gate_sb.tile([BS, 512], F32, tag="t2")
                nc.vector.tensor_add(t2b[:], t1b[:], PB[:, 1024:1536])
                nb = gate_sb.tile([BS, 512], F32, tag="n")
                nc.scalar.activation(nb[:], t2b[:], tanh)
                zm1b = gate_sb.tile([BS, 512], F32, tag="d")
                nc.vector.tensor_scalar(
                    zm1b[:], rzb[:, 512:1024], -1.0, 1.0, op0=OP.mult, op1=OP.add)
                ctx_b = h_sb.tile([BS, H], F32, tag="h")
                nc.vector.tensor_mul(ctx_b[:], zm1b[:], nb[:])

                # context_T [1024, 64] chunks: ctx_f (4) then ctx_b (4)
                transpose_copy(ctxT[:, 0:4, :], ctx_acc[:])
                transpose_copy(ctxT[:, 4:8, :], ctx_b[:])

            # ================= fc1 / fc2 / gi_dec / decoder / re_emb ========
            with (
                tc.tile_pool(name="dec_w", bufs=1) as dec_w,
                tc.tile_pool(name="dec_ps", bufs=2, space="PSUM") as dec_ps,
            ):
                fc1T_t = dec_w.tile([128, 8, L], F32R)
                for k in range(8):
                    nc.sync.dma_start(
                        fc1T_t[:, k, :], din["fc1T"][k * 128:(k + 1) * 128, :])
                fc1b_t = dec_w.tile([1, L], F32R)
                nc.sync.dma_start(fc1b_t[:], din["fc1b"])
                fc2T_t = dec_w.tile([128, D], F32R)
                nc.sync.dma_start(fc2T_t[:], din["fc2T"])
                fc2b_t = dec_w.tile([1, D], F32R)
                nc.sync.dma_start(fc2b_t[:], din["fc2b"])
                WihdT_t = dec_w.tile([128, 4, 3 * D], F32R)
                for k in range(4):
                    nc.sync.dma_start(
                        WihdT_t[:, k, :], din["WihdT"][k * 128:(k + 1) * 128, :])
                bihd_t = dec_w.tile([1, 3 * D], F32R)
                nc.sync.dma_start(bihd_t[:], din["bihd"])
                Wrzd = dec_w.tile([128, 4, 1024], F32R)
                for k in range(4):
                    nc.sync.dma_start(
                        Wrzd[:, k, :], din["Wrz_dec"][k * 128:(k + 1) * 128, :])
                Wnhd = dec_w.tile([128, 4, 512], F32R)
                for k in range(4):
                    nc.sync.dma_start(
                        Wnhd[:, k, :], din["Wnh_dec"][k * 128:(k + 1) * 128, :])
                brzd = dec_w.tile([1, 1024], F32R)
                nc.sync.dma_start(brzd[:], din["brz_dec"])
                bhnd = dec_w.tile([1, 512], F32R)
                nc.sync.dma_start(bhnd[:], din["bhn_dec"])
                rewT_t = dec_w.tile([128, 4, E], F32R)
                for k in range(4):
                    nc.sync.dma_start(
                        rewT_t[:, k, :], din["rewT"][k * 128:(k + 1) * 128, :])
                reb_t = dec_w.tile([1, E], F32R)
                nc.sync.dma_start(reb_t[:], din["reb"])

                # z = context @ fc1_w.T + fc1_b
                PZ = dec_ps.tile([BS, L], F32, tag="g")
                bias_mm(PZ[:], fc1b_t[:])
                for k in range(8):
                    nc.tensor.matmul(
                        PZ[:], ctxT[:, k, :], fc1T_t[:, k, :],
                        start=False, stop=(k == 7))
                z_sb = gate_sb.tile([BS, L], F32, tag="zz")
                nc.vector.tensor_copy(z_sb[:], PZ[:])
                nc.sync.dma_start(z_o, z_sb[:])
                zT_ps = tp_ps.tile([128, 4 * BS], F32, tag="tp")
                nc.tensor.transpose(zT_ps[:L, :BS], z_sb[:], ident[:BS, :BS])
                zT = hT_sb.tile([L, BS], F32R, tag="zT")
                nc.scalar.copy(zT[:], zT_ps[:L, :BS])

                # out0 = z @ fc2_w.T + fc2_b
                PO = dec_ps.tile([BS, D], F32, tag="g")
                bias_mm(PO[:], fc2b_t[:])
                nc.tensor.matmul(PO[:], zT[:], fc2T_t[:], start=False, stop=True)
                out0_B = h_sb.tile([BS, D], F32, tag="h")
                nc.vector.tensor_copy(out0_B[:], PO[:])
                transpose_copy(out0T[:], out0_B[:])

                # gi_dec = out0 @ dec_Wih.T + dec_bih
                PG = dec_ps.tile([BS, 3 * D], F32, tag="g")
                for nn in range(3):
                    bias_mm(PG[:, nn * 512:(nn + 1) * 512],
                            bihd_t[:, nn * 512:(nn + 1) * 512])
                for k in range(4):
                    for nn in range(3):
                        nc.tensor.matmul(
                            PG[:, nn * 512:(nn + 1) * 512],
                            out0T[:, k, :], WihdT_t[:, k, nn * 512:(nn + 1) * 512],
                            start=False, stop=(k == 3),
                        )
                nc.scalar.copy(gi_dec[:], PG[:])

                # ---------------- decoder ----------------
                h_B = out0_B
                h_T = [out0T[:, k, :] for k in range(4)]
                for t in range(T):
                    PD = dec_ps.tile([BS, 1536], F32, tag="g")
                    bias_mm(PD[:, 0:512], brzd[:, 0:512])
                    bias_mm(PD[:, 512:1024], brzd[:, 512:1024])
                    bias_mm(PD[:, 1024:1536], bhnd[:])
                    for k in range(4):
                        for nn in range(2):
                            nc.tensor.matmul(
                                PD[:, nn * 512:(nn + 1) * 512],
                                h_T[k], Wrzd[:, k, nn * 512:(nn + 1) * 512],
                                start=False, stop=(k == 3),
                            )
                        nc.tensor.matmul(
                            PD[:, 1024:1536], h_T[k], Wnhd[:, k, :],
                            start=False, stop=(k == 3),
                        )
                    rzp = gate_sb.tile([BS, 1024], F32, tag="rzp")
                    nc.vector.tensor_add(rzp[:], gi_dec[:, 0:1024], PD[:, 0:1024])
                    rz = gate_sb.tile([BS, 1024], F32, tag="rz")
                    nc.scalar.activation(rz[:], rzp[:], sigm)
                    t1 = gate_sb.tile([BS, 512], F32, tag="t1")
                    nc.vector.tensor_mul(t1[:], rz[:, 0:512], PD[:, 1024:1536])
                    t2 = gate_sb.tile([BS, 512], F32, tag="t2")
                    nc.vector.tensor_add(t2[:], t1[:], gi_dec[:, 1024:1536])
                    n_sb = gate_sb.tile([BS, 512], F32, tag="n")
                    nc.scalar.activation(n_sb[:], t2[:], tanh)
                    d = gate_sb.tile([BS, 512], F32, tag="d")
                    nc.vector.tensor_sub(d[:], h_B[:], n_sb[:])
                    e = gate_sb.tile([BS, 512], F32, tag="e")
                    nc.vector.tensor_mul(e[:], rz[:, 512:1024], d[:])
                    h_new = h_sb.tile([BS, D], F32, tag="h")
                    nc.vector.tensor_add(h_new[:], n_sb[:], e[:])

                    tsl = slice(t * BS, (t + 1) * BS)
                    transpose_copy(outs_T[:, :, tsl], h_new[:])
                    h_T = [outs_T[:, k, tsl] for k in range(4)]
                    h_B = h_new

                # ---------------- re_emb ----------------
                for m in range(TB // 128):
                    msl = slice(m * 128, (m + 1) * 128)
                    PR = dec_ps.tile([128, E], F32, tag="g")
                    nc.tensor.matmul(
                        PR[:], ones128[:], reb_t[:], start=True, stop=False)
                    for k in range(4):
                        nc.tensor.matmul(
                            PR[:], outs_T[:, k, msl], rewT_t[:, k, :],
                            start=False, stop=(k == 3),
                        )
                    # exact leaky relu: max(x, alpha*x)
                    re_a = gate_sb.tile([128, E], F32, tag="rea")
                    nc.vector.tensor_scalar_mul(re_a[:], PR[:], NEG_SLOPE)
                    re_sb = gate_sb.tile([128, E], F32, tag="re")
                    nc.vector.tensor_max(re_sb[:], PR[:], re_a[:])
                    nc.sync.dma_start(re_o[msl, :], re_sb[:])

        # ================= logits =================
        if phases in ("all", "logits"):
            if phases == "logits":
                zsrc = persist.tile([128, 4, TB], F32)
                nc.vector.memset(zsrc[:], 0.01)
                nc.scalar.copy(outs_T[:], zsrc[:])
            with (
                tc.tile_pool(name="hv_w", bufs=3) as hv_w,
                tc.tile_pool(name="stage", bufs=4) as stage_p,
                tc.tile_pool(name="lg_ps", bufs=2, space="PSUM") as lg_ps,
            ):
                for c0, clen in V_CHUNKS:
                    hv_t = hv_w.tile([128, 4, VCH], F32R, tag="hv")
                    for k in range(4):
                        nc.sync.dma_start(
                            hv_t[:, k, :clen],
                            din["hvT"][k * 128:(k + 1) * 128, c0:c0 + clen])
                    hvb_t = hv_w.tile([128, VCH], F32, tag="hvb")
                    nc.sync.dma_start(
                        hvb_t[:, :clen], din["hvb_bc"][:, c0:c0 + clen])
                    for m in range(TB // 128):
                        msl = slice(m * 128, (m + 1) * 128)
                        P = lg_ps.tile([128, VCH], F32, tag="lg")
                        for nt in range(0, clen, 512):
                            ntl = min(512, clen - nt)
                            for k in range(4):
                                nc.tensor.matmul(
                                    P[:, nt:nt + ntl],
                                    outs_T[:, k, msl], hv_t[:, k, nt:nt + ntl],
                                    start=(k == 0), stop=(k == 3),
                                )
                        st = stage_p.tile([128, VCH], F32, tag="st")
                        nc.vector.scalar_tensor_tensor(
                            st[:, :clen], P[:, :clen], 1.0, hvb_t[:, :clen],
                            op0=OP.mult, op1=OP.add)
                        nc.sync.dma_start(
                            logits_o[msl, c0:c0 + clen], st[:, :clen])


def _pack_host(inputs):
    """Full-model host prep. Returns per-core input dicts."""
    f32 = np.float32
    emb = np.asarray(inputs["emb"], f32)
    x = np.asarray(inputs["x"])
    x_lens = np.asarray(inputs["x_lens"])

    WihfT = np.ascontiguousarray(np.asarray(inputs["enc_Wih_f"], f32).T)  # [E,3H]
    WhhfT = np.ascontiguousarray(np.asarray(inputs["enc_Whh_f"], f32).T)  # [H,3H]
    bihf = np.asarray(inputs["enc_bih_f"], f32)
    bhhf = np.asarray(inputs["enc_bhh_f"], f32)
    WihbT = np.ascontiguousarray(np.asarray(inputs["enc_Wih_b"], f32).T)
    bihb = np.asarray(inputs["enc_bih_b"], f32)
    bhhb = np.asarray(inputs["enc_bhh_b"], f32)
    fc1_w = np.asarray(inputs["fc1_w"], f32)
    fc1_b = np.asarray(inputs["fc1_b"], f32)
    fc2_w = np.asarray(inputs["fc2_w"], f32)
    fc2_b = np.asarray(inputs["fc2_b"], f32)
    WihdT = np.ascontiguousarray(np.asarray(inputs["dec_Wih"], f32).T)
    WhhdT = np.ascontiguousarray(np.asarray(inputs["dec_Whh"], f32).T)
    bihd = np.asarray(inputs["dec_bih"], f32)
    bhhd = np.asarray(inputs["dec_bhh"], f32)
    rewT = np.ascontiguousarray(np.asarray(inputs["re_w"], f32).T)
    re_b = np.asarray(inputs["re_b"], f32)
    hvT = np.ascontiguousarray(np.asarray(inputs["hv_w"], f32).T)
    hv_b = np.asarray(inputs["hv_b"], f32)

    shared = {
        "Wrz_enc": np.vstack([WhhfT[:, :1024], WihfT[:, :1024]]),
        "Wnx_enc": WihfT[:, 1024:],
        "Wnh_enc": WhhfT[:, 1024:],
        "brz_enc": (bihf + bhhf)[None, :1024],
        "binn_enc": bihf[None, 1024:],
        "bhn_enc": bhhf[None, 1024:],
        "Wb_enc": WihbT,
        "bb_enc": (bihb + np.concatenate([bhhb[:1024], np.zeros(512, f32)]))[None],
        "bhn_b": np.tile(bhhb[None, 1024:], (BS, 1)),
        "fc1T": fc1_w.T,
        "fc1b": fc1_b[None],
        "fc2T": fc2_w.T,
        "fc2b": fc2_b[None],
        "WihdT": WihdT,
        "bihd": bihd[None],
        "Wrz_dec": WhhdT[:, :1024],
        "Wnh_dec": WhhdT[:, 1024:],
        "brz_dec": bhhd[None, :1024],
        "bhn_dec": bhhd[None, 1024:],
        "rewT": rewT,
        "reb": re_b[None],
        "hvT": hvT,
        "hvb_bc": np.broadcast_to(hv_b[None], (128, V)),
    }
    shared = {k: np.ascontiguousarray(v, dtype=f32) for k, v in shared.items()}

    idx = (np.asarray(x_lens) - 1).astype(np.int64)
    per_core = []
    for s in range(N_CORES):
        rows = slice(s * BS, (s + 1) * BS)
        xs = np.asarray(x[rows], np.int64)
        emb_s = emb[xs]                    # [BS, T, E]
        xembT = np.ascontiguousarray(
            emb_s.transpose(2, 1, 0).reshape(E, TB), dtype=f32)
        idx_s = idx[rows]
        xlastT = np.ascontiguousarray(
            emb_s[np.arange(BS), idx_s].T, dtype=f32)      # [E, BS]
        mask = np.zeros((BS, T), f32)
        mask[np.arange(BS), idx_s] = 1.0
        d = dict(shared)
        d.update(xembT=xembT, xlastT=xlastT, mask=mask)
        per_core.append(d)
    return per_core


def kernel(**inputs):
    if "nc" not in _COMPILED:
        _COMPILED["nc"] = build_nc()
    nc = _COMPILED["nc"]

    in_maps = _pack_host(inputs)
    res = run_bass_kernel_spmd(nc, in_maps, core_ids=list(range(N_CORES)))

    logits = np.concatenate(
        [r["logits_o"].reshape(T, BS, V) for r in res.results], axis=1)
    z = np.concatenate([r["z_o"] for r in res.results], axis=0)
    re_emb = np.concatenate(
        [r["re_o"].reshape(T, BS, E).transpose(1, 0, 2) for r in res.results],
        axis=0)
    return logits, z, re_emb
